# revision 16
# baseline (speedup 1.0000x reference)
"""Trainium2 Bass kernel for nn_ExploratoryMechanism (retrieval_knn).

Computes, for each query: latent projection qp = q @ Wq.T + bq, gated fusion of
euclidean distances to context/memory banks, and top-5 smallest fused distances
(values + indices), matching the jax reference.

Sharding: data-parallel over queries. 8 cores, each handles 1024 queries
(core k -> batch k//2, s-half k%2) against its batch's full 4096 candidates.
Indices are global per batch, so no cross-core reduction is needed.

Device pipeline per 128-query tile (q-tile), per 512-candidate slice:
  PE   : psum = -2*qc (2 fp32/fp32r matmuls, K=128 each)
         psum += c2 (one K=3 bf16 matmul vs host-precomputed bf16 triple of
         sum(c^2); ones stationary)
  ACT  : S = sqrt(psum * g^2 + q2*g^2) = g * dist   (per-partition scale/bias
         APs; gate folded into the sqrt => fused distance is a plain sum)
  Pool : m' = S_ctx + S_mem                        (gpsimd tensor_tensor)
  ACT  : m = -m'                                   (negate for max8)
  DVE  : max8(m) -> top-8 largest of -fused = top-8 smallest fused;
         max_index -> indices; negate values; stage for output.

All host-side prep (transposes to put d on partitions, c2 computation) is in
kernel() as numpy, outside the measured device execution.

Hard constraint discovered on this stack: every non-matmul instruction may
carry at most ONE sync wait (walrus setupSyncWait limit), and matmuls with
fp32/fp32r operands (self-loading weights) also only one; bf16 matmuls may
carry several. The instruction streams below are ordered so that each engine
consumes at most one new semaphore lane per instruction, with tiny "observer"
ops where a second lane would otherwise be needed. The stock TileContext tail
drain aggregates one wait per outstanding lane, so SplitDrainTileContext
splits it into single-wait drains.
"""

import sys

if "/opt/trn_rl_repo" not in sys.path:
    sys.path.insert(0, "/opt/trn_rl_repo")

import numpy as np
import ml_dtypes

import concourse.bass as bass
import concourse.mybir as mybir
from concourse.tile import TileContext
from concourse.bass_utils import run_bass_kernel_spmd
from concourse.vector_clock import ScopedClock

# problem shape (hardcoded per harness contract)
B, S, N, D = 4, 2048, 4096, 256
TOP_N = 5
N_CORES = 8
SQ = S // 2          # queries per core = 1024
QT = SQ // 128       # q-tiles per core = 8
NS = N // 512        # candidate slices = 8

F32 = mybir.dt.float32
F32R = mybir.dt.float32r
BF16 = mybir.dt.bfloat16
U32 = mybir.dt.uint32

USE_F32R = True  # distance matmuls in float32r (1 cyc/row) vs float32 (4 cyc/row)


class SplitDrainTileContext(TileContext):
    """Tail drain can only carry one sync wait; split it into a chain."""

    def _drain_and_barrier(self, tick_clock, wait_clock):
        drain_inst = self.nc.sync.drain()
        wait_clock.add_sem_waits(
            drain_inst.ins, ScopedClock({None: tick_clock.global_clock})
        )
        si = drain_inst.ins.sync_info
        if si is not None and len(si.on_wait) > 1:
            waits = list(si.on_wait)
            drain_inst.ins.sync_info = mybir.SyncInfo(
                on_wait=waits[:1], on_update=list(si.on_update)
            )
            for w in waits[1:]:
                extra = self.nc.sync.drain()
                extra.ins.sync_info = mybir.SyncInfo(on_wait=[w], on_update=[])
        self.nc.all_engine_barrier()
        assert self.sems is not None
        popped = self.nc._tile_sem_poison_stack.pop()
        assert popped is self._sem_poison
        self.nc.clear_and_free_semaphores(list(self.sems.allocated().values()))
        self.nc.all_engine_barrier()




_PROC_IDX_TO_SEM_PREFIX = {
    0: "Pool", 1: "Activation", 2: "PE", 3: "DVE", 4: "SP",
}


_STRIP_MARGIN = 12


def _strip_same_proc_waits(nc):
    """Drop *distant* semaphore waits on an instruction's own engine lane.

    Engines issue in order but pipeline adjacent instructions, so a
    same-engine hazard a few instructions back still needs its semaphore.
    Hazards >= _STRIP_MARGIN instructions back are safe: the engine queues
    are 8-deep strict FIFO, so the producer has fully retired (writes acked)
    before the consumer issues. Tile emits these distant waits mechanically
    for tile-slot releases; this walrus build rejects any non-matmul
    instruction carrying more than one sync wait, so we strip the provably
    redundant ones.
    """
    for f in nc.m.functions:
        for bb in f.blocks:
            for inst in bb.instructions:
                si = inst.sync_info
                if si is None or not si.on_wait:
                    continue
                proc = inst.bass_scheduled_proc
                prefix = _PROC_IDX_TO_SEM_PREFIX.get(proc)
                if prefix is None:
                    continue
                tick = inst.bass_scheduled_tick
                if tick is None:
                    continue
                # PE completes strictly in order (pc-monotone ends), so its
                # same-proc waits are redundant at any distance.
                margin = 0 if prefix == "PE" else _STRIP_MARGIN
                kept = [
                    w for w in si.on_wait
                    if not ((w.ant_name or "").startswith(prefix + "_")
                            and tick - w.wait_value >= margin)
                ]
                if len(kept) != len(si.on_wait):
                    inst.sync_info = mybir.SyncInfo(
                        on_wait=kept, on_update=list(si.on_update))


def build_kernel(debug_dump=False):
    nc = bass.Bass()
    dt_mm = F32R if USE_F32R else F32

    # columns 0..SQ-1 = qT, SQ..SQ+D-1 = WqT ([d_in, e])
    qw = nc.declare_dram_parameter("qw", [D, SQ + D], dt_mm, isOutput=False)
    # columns 0..N-1 = ctxT, N..2N-1 = memT
    cm = nc.declare_dram_parameter("cm", [D, 2 * N], dt_mm, isOutput=False)
    # consts columns: 0-1 bq (k-tiles), 2-3 wg (k-tiles), 4 bg, 5 ones
    consts = nc.declare_dram_parameter("consts", [128, 6], F32, isOutput=False)
    # c2 triples (hi, mid, lo) per array
    c2ctx = nc.declare_dram_parameter("c2ctx", [3, N], BF16, isOutput=False)
    c2mem = nc.declare_dram_parameter("c2mem", [3, N], BF16, isOutput=False)
    out_vals = nc.declare_dram_parameter("out_vals", [SQ, TOP_N], F32, isOutput=True)
    out_idx = nc.declare_dram_parameter("out_idx", [SQ, TOP_N], U32, isOutput=True)
    if debug_dump:
        dbg = nc.declare_dram_parameter("dbg", [128, 2048 + 1024 + 48], F32, isOutput=True)

    with SplitDrainTileContext(nc) as tc:
        with (
            tc.tile_pool(name="persist", bufs=1) as pp,
            tc.tile_pool(name="work", bufs=1) as wp,
        ):
            # ---------------- loads (5 DMAs, one lane each) ----------------
            qw_t = pp.tile([128, 2, SQ + D], dt_mm, tag="qw")
            cm_t = pp.tile([128, 2, 2 * N], dt_mm, tag="cm")
            cst = pp.tile([128, 6], F32, tag="cst")
            c2c_t = pp.tile([3, N], BF16, tag="c2c")
            c2m_t = pp.tile([3, N], BF16, tag="c2m")
            nc.gpsimd.dma_start(out=qw_t[:], in_=qw.rearrange("(k p) s -> p k s", p=128))
            nc.gpsimd.dma_start(out=cm_t[:], in_=cm.rearrange("(k p) n -> p k n", p=128))
            nc.gpsimd.dma_start(out=cst[:], in_=consts[:])
            nc.gpsimd.dma_start(out=c2c_t[:], in_=c2ctx[:])
            nc.gpsimd.dma_start(out=c2m_t[:], in_=c2mem[:])

            qT_view = qw_t[:, :, 0:SQ]
            wq_view = qw_t[:, :, SQ:SQ + D]
            ctx_view = cm_t[:, :, 0:N]
            mem_view = cm_t[:, :, N:2 * N]

            ones3 = pp.tile([3, 128], BF16, tag="ones3")
            nc.gpsimd.memset(ones3[:], 1.0)

            # ------------- PE fence: absorb the 6 DMA lanes ----------------
            with tc.tile_pool(name="fence_ps", bufs=1, space="PSUM") as fps:
                f = fps.tile([128, 16], F32, tag="f")
                nc.tensor.matmul(
                    f[0:1, 0:1],
                    qw_t[:, 0, 0:1].bitcast(BF16)[0:1, 0:1],
                    cm_t[:, 0, 0:1].bitcast(BF16)[0:1, 0:1],
                    start=True, stop=True,
                )
                nc.tensor.matmul(
                    f[0:1, 0:1],
                    cst[:, 0:1].bitcast(BF16)[0:1, 0:1],
                    c2c_t[0:1, 0:1],
                    start=True, stop=True,
                )
                nc.tensor.matmul(
                    f[0:1, 0:1],
                    ones3[0:1, 0:1],
                    c2m_t[0:1, 0:1],
                    start=True, stop=True,
                )

            # ---------------- prep: qp projection, gate, q2 ----------------
            qp_t = pp.tile([128, 2, SQ], dt_mm, tag="qp")   # qp transposed [e, s]
            g_col = pp.tile([128, QT], F32, tag="gcol")     # gate per query
            q2col = pp.tile([128, QT], F32, tag="q2col")    # |qp|^2 per query
            sA = pp.tile([128, QT], F32, tag="sA")          # g^2
            sB = pp.tile([128, QT], F32, tag="sB")          # (1-g)^2
            bA = pp.tile([128, QT], F32, tag="bA")          # q2*g^2
            bB = pp.tile([128, QT], F32, tag="bB")          # q2*(1-g)^2
            omg = pp.tile([128, QT], F32, tag="omg")        # 1-g
            act_scr = pp.tile([1, 4], F32, tag="ascr")      # ACT observer scratch

            # ACT observer: bring the consts DMA lane onto ACT's clock
            nc.scalar.activation(act_scr[0:1, 0:1], cst[0:1, 0:1],
                                 mybir.ActivationFunctionType.Copy)

            with tc.tile_pool(name="prep_ps", bufs=1, space="PSUM") as prep:
                # projection: qpT[e, s] += wqT[d, e].T @ qT[d, s]
                ps_qp = []
                for i in range(4):
                    ps_qp_i = prep.tile([128, 512], F32, tag=f"psqp{i}")
                    ps_qp.append(ps_qp_i)
                for e in range(2):
                    for s in range(2):
                        for k in range(2):
                            nc.tensor.matmul(
                                ps_qp[e * 2 + s][:],
                                wq_view[:, k, e * 128:(e + 1) * 128].bitcast(F32),
                                qT_view[:, k, s * 512:(s + 1) * 512].bitcast(F32),
                                start=(k == 0), stop=(k == 1),
                            )
                for e in range(2):
                    for s in range(2):
                        # qp stored pre-scaled by -2 so the distance psum
                        # accumulates -2*qc directly; q2 is rescaled by 1/4.
                        nc.scalar.activation(
                            qp_t[:, e, s * 512:(s + 1) * 512],
                            ps_qp[e * 2 + s][:],
                            mybir.ActivationFunctionType.Identity,
                            bias=cst[:, e:e + 1],
                            scale=-2.0,
                        )

                # gate: z[t-col] = qT.T @ wg (+bg) -> sigmoid
                ps_z = prep.tile([128, QT], F32, tag="psz")
                for t in range(QT):
                    for k in range(2):
                        nc.tensor.matmul(
                            ps_z[:, t:t + 1],
                            qT_view[:, k, t * 128:(t + 1) * 128].bitcast(F32),
                            cst[:, 2 + k:3 + k],
                            start=(k == 0), stop=(k == 1),
                        )
                nc.scalar.activation(g_col[:], ps_z[:],
                                     mybir.ActivationFunctionType.Sigmoid,
                                     bias=cst[:, 4:5])

                # q2 = sum over e of qp^2, per query column
                sq_t = wp.tile([128, 2, SQ], F32, tag="sq")
                for e in range(2):
                    nc.scalar.activation(sq_t[:, e, :], qp_t[:, e, :].bitcast(F32),
                                         mybir.ActivationFunctionType.Square)
                ps_q2 = prep.tile([128, QT], F32, tag="psq2")
                for t in range(QT):
                    for k in range(2):
                        nc.tensor.matmul(
                            ps_q2[:, t:t + 1],
                            sq_t[:, k, t * 128:(t + 1) * 128],
                            cst[:, 5:6],
                            start=(k == 0), stop=(k == 1),
                        )
                nc.scalar.activation(q2col[:], ps_q2[:],
                                     mybir.ActivationFunctionType.Copy,
                                     scale=0.25)

            # per-partition scale/bias tiles, all on ACT (same-engine, no waits)
            nc.scalar.activation(sA[:], g_col[:], mybir.ActivationFunctionType.Square)
            nc.scalar.activation(omg[:], g_col[:], mybir.ActivationFunctionType.Copy,
                                 bias=1.0, scale=-1.0)
            nc.scalar.activation(sB[:], omg[:], mybir.ActivationFunctionType.Square)
            for t in range(QT):
                nc.scalar.activation(bA[:, t:t + 1], q2col[:, t:t + 1],
                                     mybir.ActivationFunctionType.Copy,
                                     scale=sA[:, t:t + 1])
                nc.scalar.activation(bB[:, t:t + 1], q2col[:, t:t + 1],
                                     mybir.ActivationFunctionType.Copy,
                                     scale=sB[:, t:t + 1])

            # one-time ACT observer: wait for the last prep-produced tile so
            # every later sqrt's (ACT, prep-tick) requirement is dominated
            nc.scalar.activation(act_scr[0:1, 2:3], bB[0:1, QT - 1:QT],
                                 mybir.ActivationFunctionType.Copy)

            # ---------------- distance + top-k main loop ------------------
            if debug_dump:
                dbg_t = pp.tile([128, 2048 + 1024 + 48], F32, tag="dbgall")
            vals_stage = pp.tile([128, QT, 8], F32, tag="vstage")
            idx_stage = pp.tile([128, QT, 8], U32, tag="istage")

            with (
                tc.tile_pool(name="dist_ps", bufs=8, space="PSUM") as dps,
                tc.tile_pool(name="sab", bufs=10) as sab_pool,
                tc.tile_pool(name="mprime", bufs=16) as mp_pool,
                tc.tile_pool(name="mfull", bufs=2) as m_pool,
            ):
                for t in range(QT):
                    m_t = m_pool.tile([128, N], F32, tag="m")
                    for j in range(NS):
                        sl = slice(j * 512, (j + 1) * 512)
                        chunks = []
                        for arr_i, arr in enumerate((ctx_view, mem_view)):
                            ps = dps.tile([128, 512], F32, tag="ps")
                            for k in range(2):
                                nc.tensor.matmul(
                                    ps[:],
                                    qp_t[:, k, t * 128:(t + 1) * 128],
                                    arr[:, k, sl],
                                    start=(k == 0), stop=False,
                                )
                            nc.tensor.matmul(
                                ps[:], ones3[:],
                                (c2c_t if arr_i == 0 else c2m_t)[:, sl],
                                start=False, stop=True,
                            )
                            s_ab = sab_pool.tile([128, 512], F32, tag="sab")
                            nc.scalar.activation(
                                s_ab[:], ps[:], mybir.ActivationFunctionType.Sqrt,
                                bias=(bA if arr_i == 0 else bB)[:, t:t + 1],
                                scale=(sA if arr_i == 0 else sB)[:, t:t + 1],
                            )
                            chunks.append(s_ab)
                        if debug_dump and t == 0 and j == 0:
                            nc.scalar.copy(dbg_t[:, 2048:2048 + 512], chunks[0][:])
                            nc.scalar.copy(dbg_t[:, 2048 + 512:2048 + 1024], chunks[1][:])
                        mp = mp_pool.tile([128, 512], F32, tag="mp")
                        nc.gpsimd.tensor_tensor(mp[:], chunks[0][:], chunks[1][:],
                                                mybir.AluOpType.add)
                        nc.scalar.activation(m_t[:, sl], mp[:],
                                             mybir.ActivationFunctionType.Copy,
                                             scale=-1.0)
                    # top-8 smallest fused = top-8 largest of m = -fused
                    vneg = wp.tile([128, 8], F32, tag=f"vneg{t}")
                    nc.vector.max(vneg[:], m_t[:])
                    nc.vector.max_index(idx_stage[:, t, :], vneg[:], m_t[:])
                    nc.vector.tensor_scalar_mul(vals_stage[:, t, :], vneg[:], -1.0)
                    if debug_dump and t == 0:
                        nc.scalar.copy(dbg_t[:, 0:2048], m_t[:, 0:2048])
                    # ACT observer: bring DVE's clock onto ACT for m-slot WAR reuse
                    obs_t = wp.tile([1, 2], F32, tag=f"obs{t}")
                    nc.scalar.activation(obs_t[0:1, 0:1],
                                         vals_stage[0:1, t, 0:1],
                                         mybir.ActivationFunctionType.Copy)

            # ---------------- outputs (2 DMAs) ----------------------------
            if debug_dump:
                for i, src_t in enumerate((g_col, q2col, sA, sB, bA, bB)):
                    nc.scalar.copy(dbg_t[:, 3072 + i * 8:3072 + (i + 1) * 8], src_t[:])
                nc.gpsimd.dma_start(out=dbg[:], in_=dbg_t[:])
            pobs = pp.tile([1, 2], U32, tag="pobs")
            nc.gpsimd.tensor_copy(pobs[0:1, 0:1], idx_stage[0:1, 0, 0:1])
            nc.gpsimd.dma_start(
                out=out_vals.rearrange("(t p) f -> p t f", p=128),
                in_=vals_stage[:, :, 0:TOP_N],
            )
            nc.gpsimd.dma_start(
                out=out_idx.rearrange("(t p) f -> p t f", p=128),
                in_=idx_stage[:, :, 0:TOP_N],
            )

    _strip_same_proc_waits(nc)
    return nc


_NC_CACHE = None


def _get_nc():
    global _NC_CACHE
    if _NC_CACHE is None:
        _NC_CACHE = build_kernel()
    return _NC_CACHE


def _split3_bf16(x: np.ndarray):
    """Split fp32 vector into three bf16 components summing to ~x (24 bits)."""
    hi = x.astype(ml_dtypes.bfloat16)
    r1 = x - hi.astype(np.float32)
    mid = r1.astype(ml_dtypes.bfloat16)
    lo = (r1 - mid.astype(np.float32)).astype(ml_dtypes.bfloat16)
    return hi, mid, lo


def make_in_maps(query_embeddings, context_embeddings, memory_embeddings,
                 Wq, bq, wg, bg):
    q = np.asarray(query_embeddings, dtype=np.float32)
    ctx = np.asarray(context_embeddings, dtype=np.float32)
    mem = np.asarray(memory_embeddings, dtype=np.float32)
    Wq = np.asarray(Wq, dtype=np.float32)
    bq = np.asarray(bq, dtype=np.float32)
    wg = np.asarray(wg, dtype=np.float32)
    bg = np.asarray(bg, dtype=np.float32)

    wqT = Wq.T  # [d_in, e]
    consts = np.zeros((128, 6), dtype=np.float32)
    consts[:, 0] = -2.0 * bq[0:128]
    consts[:, 1] = -2.0 * bq[128:256]
    consts[:, 2] = wg[0:128]
    consts[:, 3] = wg[128:256]
    consts[:, 4] = bg[0]
    consts[:, 5] = 1.0

    in_maps = []
    for core in range(N_CORES):
        b = core // 2
        half = core % 2
        qs = q[b, half * SQ:(half + 1) * SQ]            # [1024, 256]
        qw_k = np.concatenate([qs.T, wqT], axis=1)      # [256, 1024+256]
        cm_k = np.concatenate([ctx[b].T, mem[b].T], axis=1)  # [256, 8192]
        c2c = (ctx[b].astype(np.float64) ** 2).sum(-1).astype(np.float32)
        c2m = (mem[b].astype(np.float64) ** 2).sum(-1).astype(np.float32)
        in_maps.append({
            "qw": np.ascontiguousarray(qw_k),
            "cm": np.ascontiguousarray(cm_k),
            "consts": consts,
            "c2ctx": np.stack(_split3_bf16(c2c), axis=0),
            "c2mem": np.stack(_split3_bf16(c2m), axis=0),
        })
    return in_maps


def kernel(query_embeddings, context_embeddings, memory_embeddings,
           Wq, bq, wg, bg, _trace=False):
    nc = _get_nc()
    in_maps = make_in_maps(query_embeddings, context_embeddings,
                           memory_embeddings, Wq, bq, wg, bg)
    res = run_bass_kernel_spmd(nc, in_maps, list(range(N_CORES)), trace=_trace)

    vals = np.empty((B, S, TOP_N), dtype=np.float32)
    idx = np.empty((B, S, TOP_N), dtype=np.int32)
    for core in range(N_CORES):
        b = core // 2
        half = core % 2
        r = res.results[core]
        vals[b, half * SQ:(half + 1) * SQ] = r["out_vals"]
        idx[b, half * SQ:(half + 1) * SQ] = r["out_idx"].view(np.int32)
    if _trace:
        return (vals, idx), res
    return (vals, idx)
